# revision 12
# baseline (speedup 1.0000x reference)
"""Self-contained Trainium2 Bass kernel for a 2-layer length-masked LSTM encoder.

Model (matches the reference):
    x = embedding[tokens]                       # [B, T, H]
    for l in range(2): x, (c, h) = lstm_layer(x, lengths, Wx[l], Wh[l], b[l])
    return x, (c, h)

Strategy: data-parallel over batch across 8 cores (B=32 -> 4 seqs/core).
Each core runs both LSTM layers for its 4 sequences with zero cross-core
communication.  Layers are software-pipelined with a 16-step delay so both
layers' serial chains interleave on the engines.  (Tensor-parallel gate
splitting was evaluated: it needs a per-step cross-core h broadcast, and
neither remote_dma nor the kernel barrier is functional under this axon
PJRT runtime; ncfw collectives have a ~5us/call floor - 512 calls would
dominate.  The kernel sits at the PE weight-ingest floor: 2 layers x 64
LDWEIGHTS x ~53ns x 256 steps ~= 1.74ms, which matches measurement.)

Key layout trick ("cell-partition"): all per-step tensors are stored as
[128 cell-partitions, cell-block, batch] so that
  - gate M-tiles are direct [128,128] slices of the weight matrices,
  - every elementwise op pairs operands on the same partitions,
  - the h history [128, kchunk, t, b] is directly the rhs of the next
    step's matmul (no transposes in the hot loop).

Length masking is removed from the hot loop entirely: the recurrence runs
unmasked.  Frozen state at t >= len only depends on steps < len, so the
graded outputs are reconstructed afterwards (tail-fill of ys with
ys[b, len-1]; final (c, h) read from history at step len-1).
"""

import numpy as np

# Hyperparameters of the problem (hardcoded per the harness contract).
B, T, V, H, L = 32, 256, 32000, 512, 2
NCORES = 8
BL = B // NCORES          # batch per core = 4
GH = 4 * H                # gate dim = 2048
KC = H // 128             # K chunks = 4
MT = GH // 128            # M tiles  = 16 (4 gates x 4 cell blocks)
NCB = H // 128            # cell blocks per gate = 4
DT = 16                   # layer-pipeline delay / Zx block size
TMIN = T // 2             # smallest possible length

_BUILT = {}


def _build(t_steps=T):
    """Build the Bass module (one SPMD program for all cores)."""
    import concourse.bass as bass
    import concourse.mybir as mybir
    import concourse.tile as tile
    from concourse import bacc
    from concourse.masks import make_identity
    from contextlib import ExitStack

    f32 = mybir.dt.float32
    bf16 = mybir.dt.bfloat16
    i32 = mybir.dt.int32
    AF = mybir.ActivationFunctionType

    Tt = t_steps
    dt = min(DT, Tt)
    n_tok = Tt * BL                      # gathered rows per core
    n_tok_pad = ((n_tok + 127) // 128) * 128
    n_gblk = n_tok_pad // 128            # gather calls
    # c-history slots to emit: c at step te lives in ring slot te+1.
    # te in [Tt//2 - 1, Tt - 1] -> slots [Tt//2, Tt].
    ch0 = Tt // 2
    chn = Tt - ch0 + 1

    nc = bacc.Bacc("TRN2", target_bir_lowering=False, debug=False, num_devices=NCORES)

    toks = nc.declare_dram_parameter("tokens", [n_tok_pad], i32, isOutput=False)
    emb = nc.declare_dram_parameter("emb", [V, H], f32, isOutput=False)
    wx = nc.declare_dram_parameter("wx", [L, H, GH], f32, isOutput=False)
    wh = nc.declare_dram_parameter("wh", [L, H, GH], f32, isOutput=False)
    bia = nc.declare_dram_parameter("b", [L, GH], f32, isOutput=False)
    ys = nc.declare_dram_parameter("ys", [128, NCB, Tt, BL], f32, isOutput=True)
    chist = nc.declare_dram_parameter("chist", [128, chn, NCB * BL], f32, isOutput=True)

    with ExitStack() as ctx:
        tc = ctx.enter_context(tile.TileContext(nc))

        # ---- persistent pools -------------------------------------------
        wpool = ctx.enter_context(tc.tile_pool(name="weights", bufs=1))
        spool = ctx.enter_context(tc.tile_pool(name="state", bufs=1))
        zxpool = ctx.enter_context(tc.tile_pool(name="zx", bufs=2))
        work = ctx.enter_context(tc.tile_pool(name="work", bufs=3))

        # x^T in SBUF (bf16): [kchunk][128, n_tok_pad]
        XT = [spool.tile([128, n_tok_pad], bf16, tag=f"xt{k}", name=f"xt{k}") for k in range(KC)]

        # ---- embedding gather + transpose (prelude scope) ---------------
        with tc.tile_pool(name="gather", bufs=1) as gpool, \
             tc.tile_pool(name="tpsum", bufs=2, space="PSUM") as tpsum:
            toksb = gpool.tile([128, n_gblk], i32)
            nc.gpsimd.dma_start(out=toksb[:],
                                in_=toks.ap().rearrange("(j p) -> p j", p=128))
            xsb = [gpool.tile([128, H], f32, name=f"xsb{j}", tag=f"xsb{j}")
                   for j in range(n_gblk)]
            for j in range(n_gblk):
                nc.gpsimd.indirect_dma_start(
                    out=xsb[j][:], out_offset=None,
                    in_=emb[:, :],
                    in_offset=bass.IndirectOffsetOnAxis(ap=toksb[:, j:j + 1], axis=0),
                )
            idt = gpool.tile([128, 128], f32)
            make_identity(nc, idt[:])
            for j in range(n_gblk):
                for k in range(KC):
                    tp = tpsum.tile([128, 128], f32)
                    nc.tensor.transpose(out=tp[:], in_=xsb[j][:, 128 * k:128 * (k + 1)],
                                        identity=idt[:])
                    nc.vector.tensor_copy(out=XT[k][:, 128 * j:128 * (j + 1)], in_=tp[:])

        # Weights in SBUF, bf16, one [128, GH] tile per (layer, mat, kchunk).
        WSB = {}
        for l in range(L):
            for name, src in (("wx", wx), ("wh", wh)):
                for k in range(KC):
                    wt = wpool.tile([128, GH], bf16, tag=f"w{name}{l}{k}", name=f"w{name}{l}{k}")
                    # DMA with dtype cast (SWDGE): DRAM f32 -> SBUF bf16
                    nc.gpsimd.dma_start(out=wt[:], in_=src[l, 128 * k:128 * (k + 1), :])
                    WSB[(l, name, k)] = wt

        # bias -> [128, L, MT] cell-partition layout: bsb[p, l, m] = b[l, 128m+p]
        bsb = wpool.tile([128, L, MT], f32, tag="bias")
        nc.sync.dma_start(out=bsb[:], in_=bia.ap().rearrange("l (m p) -> p l m", p=128))

        # h history per layer: slot s holds h_{s-1}; slot 0 = zeros.
        HH = [spool.tile([128, KC, Tt + 1, BL], bf16, tag=f"h{l}", name=f"h{l}") for l in range(L)]
        for l in range(L):
            nc.vector.memset(HH[l][:, :, 0, :], 0.0)
        # gate/state tiles: [si | sf | so | tg | c], c is persistent state.
        GG = [spool.tile([128, 5 * 4 * BL], f32, tag=f"gg{l}", name=f"gg{l}")
              for l in range(L)]
        for l in range(L):
            nc.vector.memset(GG[l][:, 4 * 4 * BL:], 0.0)
        # layer-1 c history ring (slot s = c after step s-1), for chist.
        CR1 = spool.tile([128, Tt + 1, NCB * BL], f32, tag="c1", name="c1")

        # ---- hot loop ----------------------------------------------------
        zpsum = ctx.enter_context(tc.tile_pool(name="zpsum", bufs=2, space="PSUM"))
        zxpsum = ctx.enter_context(tc.tile_pool(name="zxpsum", bufs=2, space="PSUM"))

        ZXS = [[None, None] for _ in range(L)]  # double-buffered Zx blocks

        def zx_mtile(l, bi, m):
            """One M-tile of the Zx block bi for layer l (spread across ticks)."""
            t0 = bi * dt
            buf = bi % 2
            if m == 0:
                zt = zxpool.tile([128, MT, dt, BL], f32, tag=f"zx{l}",
                                 name=f"zx{l}_{bi}")
                ZXS[l][buf] = zt
            zt = ZXS[l][buf]
            corder = (0, 1, 3, 2)
            col = corder[m // NCB] * NCB + (m % NCB)
            zpx = zxpsum.tile([128, dt * BL], f32)
            for k in range(KC):
                rhs = (XT[k][:, t0 * BL:(t0 + dt) * BL] if l == 0
                       else HH[0][:, k, t0 + 1:t0 + dt + 1, :])
                nc.tensor.matmul(out=zpx[:], lhsT=WSB[(l, "wx", k)][:, 128 * m:128 * (m + 1)],
                                 rhs=rhs, start=(k == 0), stop=(k == KC - 1))
            nc.scalar.activation(zt[:, col, :, :],
                                 zpx[:].rearrange("p (t b) -> p t b", b=BL),
                                 AF.Identity, bias=bsb[:, l, m:m + 1])

        def zx_block(l, bi):
            for m in range(MT):
                zx_mtile(l, bi, m)

        def step(l, t):
            """One recurrence step of layer l (layer-1 lags by DL ticks)."""
            buf = (t // dt) % 2
            trel = t % dt
            G = 4 * BL  # columns per gate group = 16
            # psum column order [i, f, o, g] (gate 2<->3 swapped) so that
            # the three sigmoid gates are contiguous for one tanh(0.5 z) op.
            corder = (0, 1, 3, 2)
            zp = zpsum.tile([128, MT * BL], f32, tag=f"z{l}")
            # Preload Zx_t into the psum bank (off the serial chain); the
            # matmuls then accumulate onto it: the bank's has_written bits
            # were set once by the prelude dummy accumulation group and are
            # never cleared again (no start=True on this tag), so the PE
            # adds instead of overwriting.
            nc.vector.tensor_copy(zp[:].rearrange("p (m b) -> p m b", b=BL),
                                  ZXS[l][buf][:, :, trel, :])
            def mm_gate(g):
                for m in range(g * NCB, (g + 1) * NCB):
                    col = corder[m // NCB] * NCB + (m % NCB)
                    for k in range(KC):
                        nc.tensor.matmul(out=zp[:, BL * col:BL * (col + 1)],
                                         lhsT=WSB[(l, "wh", k)][:, 128 * m:128 * (m + 1)],
                                         rhs=HH[l][:, k, t, :],
                                         start=False, stop=(k == KC - 1),
                                         skip_group_check=True)
            gt = GG[l]  # persistent [128, 5*G]: [si | sf | so | tg | c]
            pp = work.tile([128, 2 * G], f32, tag=f"p{l}")
            th = work.tile([128, G], f32, tag=f"th{l}")
            # i, f, g gate tiles first: the c-path starts after 12/16 tiles;
            # the o tiles + fillers keep PE busy during the c chain.
            mm_gate(0), mm_gate(1), mm_gate(2)
            nc.scalar.activation(gt[:, 0:2 * G], zp[:, 0:2 * G], AF.Tanh, scale=0.5)
            nc.scalar.activation(gt[:, 3 * G:4 * G], zp[:, 3 * G:4 * G], AF.Tanh)
            nc.vector.tensor_scalar(out=gt[:, 0:2 * G], in0=gt[:, 0:2 * G],
                                    scalar1=0.5, scalar2=0.5,
                                    op0=mybir.AluOpType.mult,
                                    op1=mybir.AluOpType.add)
            mm_gate(3)
            # [si*tg | sf*c] in one packed mul, then fold
            nc.vector.tensor_mul(pp[:], gt[:, 0:2 * G], gt[:, 3 * G:5 * G])
            nc.vector.tensor_add(gt[:, 4 * G:5 * G], pp[:, 0:G], pp[:, G:2 * G])
            if l == 1:  # keep c history for final-state extraction
                nc.vector.tensor_copy(CR1[:, t + 1, :], gt[:, 4 * G:5 * G])
            nc.scalar.activation(th[:], gt[:, 4 * G:5 * G], AF.Tanh)
            # sigma(o) off the c chain
            nc.scalar.activation(gt[:, 2 * G:3 * G], zp[:, 2 * G:3 * G], AF.Tanh,
                                 scale=0.5)
            nc.vector.tensor_scalar(out=gt[:, 2 * G:3 * G], in0=gt[:, 2 * G:3 * G],
                                    scalar1=0.5, scalar2=0.5,
                                    op0=mybir.AluOpType.mult,
                                    op1=mybir.AluOpType.add)
            nc.vector.tensor_mul(HH[l][:, :, t + 1, :],
                                 gt[:, 2 * G:3 * G].rearrange("p (m b) -> p m b", b=BL),
                                 th[:].rearrange("p (m b) -> p m b", b=BL))

        # one-time has_written init for the recurrent-z psum banks:
        # a start=True zero matmul per (layer, parity-buf) slot.
        zdummy = spool.tile([128, 128], bf16, tag="zdummy", name="zdummy")
        nc.vector.memset(zdummy[:], 0.0)
        for l in range(L):
            for par in range(2):
                zp0 = zpsum.tile([128, MT * BL], f32, tag=f"z{l}")
                nc.tensor.matmul(out=zp0[:], lhsT=zdummy[:],
                                 rhs=zdummy[:, 0:MT * BL], start=True, stop=True)

        zx_block(0, 0)
        n_blocks = Tt // dt
        DL = min(2 * dt, Tt)  # layer-1 delay (2 blocks, so zx1 production
        #                       finishes one block before consumption)
        for tick in range(Tt + DL):
            # spread next zx block m-tiles, one per tick, ahead of the mms
            if tick < Tt and tick // dt + 1 < n_blocks:
                zx_mtile(0, tick // dt + 1, tick % dt)
            if tick < Tt:
                step(0, tick)
            if dt <= tick < Tt + dt and (tick - dt) // dt < n_blocks:
                zx_mtile(1, (tick - dt) // dt, tick % dt)
            if tick >= DL:
                step(1, tick - DL)

        # ---- outputs -----------------------------------------------------
        # ys = h1 history slots 1..Tt, cast bf16 -> f32 in DMA (SWDGE).
        nc.gpsimd.dma_start(out=ys[:, :, :, :], in_=HH[1][:, :, 1:Tt + 1, :])
        nc.sync.dma_start(out=chist[:, :, :], in_=CR1[:, ch0:Tt + 1, :])

    nc.compile()
    return nc


def _get_nc(t_steps=T):
    if t_steps not in _BUILT:
        _BUILT[t_steps] = _build(t_steps)
    return _BUILT[t_steps]


def _prep_inputs(tokens, lengths, embedding, Wx, Wh, b, t_steps=T):
    """Build per-core in_maps."""
    tokens = np.asarray(tokens)
    n_tok = t_steps * BL
    n_tok_pad = ((n_tok + 127) // 128) * 128
    in_maps = []
    for c in range(NCORES):
        tk = np.asarray(tokens[c * BL:(c + 1) * BL, :t_steps], dtype=np.int32)
        tk_tm = np.ascontiguousarray(tk.T).reshape(-1)  # t-major flat
        tk_pad = np.zeros([n_tok_pad], np.int32)
        tk_pad[:n_tok] = tk_tm
        in_maps.append({
            "tokens": tk_pad,
            "emb": np.asarray(embedding, np.float32),
            "wx": np.asarray(Wx, np.float32),
            "wh": np.asarray(Wh, np.float32),
            "b": np.asarray(b, np.float32),
        })
    return in_maps


def _assemble(results, lengths, t_steps=T):
    """Gather per-core outputs into the full (ys, (c, h)) pytree."""
    lengths = np.asarray(lengths).astype(np.int64)
    ch0 = t_steps // 2
    ys_full = np.zeros([B, t_steps, H], np.float32)
    c_full = np.zeros([B, H], np.float32)
    h_full = np.zeros([B, H], np.float32)
    for c in range(NCORES):
        r = results[c]
        ysc = r["ys"]          # [128, NCB, Tt, BL]
        chc = r["chist"]       # [128, chn, NCB*BL]
        # ys[b, t, mc*128+p] = ysc[p, mc, t, b]
        ysc_t = np.transpose(ysc, (3, 2, 1, 0)).reshape(BL, t_steps, H)
        ys_full[c * BL:(c + 1) * BL] = ysc_t
        chc_t = chc.reshape(128, -1, NCB, BL)
        for bloc in range(BL):
            bg = c * BL + bloc
            le = int(lengths[bg])
            # tail-fill: state frozen from step le onward
            if le < t_steps:
                ys_full[bg, le:] = ys_full[bg, le - 1]
            h_full[bg] = ys_full[bg, le - 1]
            # c at step le-1 lives at ring slot le -> chist index le - ch0
            cf = chc_t[:, le - ch0, :, bloc]          # [128, NCB]
            c_full[bg] = np.transpose(cf, (1, 0)).reshape(H)
    return ys_full, (c_full, h_full)


def _install_profile_hook():
    """Provide antenv.axon_hooks in slim images so trace=True works."""
    import sys, types, ctypes, contextlib
    try:
        import antenv.axon_hooks  # noqa
        return
    except ImportError:
        pass
    so_path = "/opt/axon/libaxon_pjrt.so"
    hook = None
    try:
        lib = ctypes.CDLL(so_path)
        if hasattr(lib, "axon_start_nrt_profile"):
            lib.axon_start_nrt_profile.argtypes = [
                ctypes.POINTER(ctypes.c_int64), ctypes.c_size_t]
            lib.axon_start_nrt_profile.restype = ctypes.c_int64
            lib.axon_stop_nrt_profile.argtypes = [ctypes.c_char_p]
            lib.axon_stop_nrt_profile.restype = ctypes.c_int64

            @contextlib.contextmanager
            def _hook(output_dir, device_ids):
                import jax
                jax.devices()
                if device_ids:
                    ids = (ctypes.c_int64 * len(device_ids))(*device_ids)
                    rc = lib.axon_start_nrt_profile(ids, len(device_ids))
                else:
                    rc = lib.axon_start_nrt_profile(None, 0)
                if rc != 0:
                    raise RuntimeError(f"axon_start_nrt_profile rc={rc}")
                try:
                    yield
                finally:
                    n = lib.axon_stop_nrt_profile(str(output_dir).encode())
                    print(f"profile: {n} file(s) written to {output_dir}")
            hook = _hook
    except OSError:
        pass
    mod = types.ModuleType("antenv.axon_hooks")
    mod.get_axon_ntff_profile_hook = lambda: hook
    mod.set_axon_ntff_profile_hook = lambda h: None
    sys.modules["antenv.axon_hooks"] = mod


def run(tokens, lengths, embedding, Wx, Wh, b, t_steps=T, trace=False):
    from concourse import bass_utils
    if trace:
        _install_profile_hook()
    nc = _get_nc(t_steps)
    in_maps = _prep_inputs(tokens, lengths, embedding, Wx, Wh, b, t_steps)
    res = bass_utils.run_bass_kernel_spmd(
        nc, in_maps, core_ids=list(range(NCORES)), trace=trace)
    out = _assemble(res.results, lengths, t_steps)
    return out, res


def kernel(tokens, lengths, embedding, Wx, Wh, b):
    out, _ = run(tokens, lengths, embedding, Wx, Wh, b)
    return out


# revision 13
# speedup vs baseline: 1.2029x; 1.2029x over previous
"""Self-contained Trainium2 Bass kernel for a 2-layer length-masked LSTM encoder.

Model (matches the reference):
    x = embedding[tokens]                       # [B, T, H]
    for l in range(2): x, (c, h) = lstm_layer(x, lengths, Wx[l], Wh[l], b[l])
    return x, (c, h)

Strategy: data-parallel over batch across 8 cores (B=32 -> 4 seqs/core).
Each core runs both LSTM layers for its 4 sequences with zero cross-core
communication.  Layers are software-pipelined with a 16-step delay so both
layers' serial chains interleave on the engines.  (Tensor-parallel gate
splitting was evaluated: it needs a per-step cross-core h broadcast, and
neither remote_dma nor the kernel barrier is functional under this axon
PJRT runtime; ncfw collectives have a ~5us/call floor - 512 calls would
dominate.  The kernel sits at the PE weight-ingest floor: 2 layers x 64
LDWEIGHTS x ~53ns x 256 steps ~= 1.74ms, which matches measurement.)

Key layout trick ("cell-partition"): all per-step tensors are stored as
[128 cell-partitions, cell-block, batch] so that
  - gate M-tiles are direct [128,128] slices of the weight matrices,
  - every elementwise op pairs operands on the same partitions,
  - the h history [128, kchunk, t, b] is directly the rhs of the next
    step's matmul (no transposes in the hot loop).

Length masking is removed from the hot loop entirely: the recurrence runs
unmasked.  Frozen state at t >= len only depends on steps < len, so the
graded outputs are reconstructed afterwards (tail-fill of ys with
ys[b, len-1]; final (c, h) read from history at step len-1).
"""

import numpy as np

# Hyperparameters of the problem (hardcoded per the harness contract).
B, T, V, H, L = 32, 256, 32000, 512, 2
NCORES = 8
BL = B // NCORES          # batch per core = 4
GH = 4 * H                # gate dim = 2048
KC = H // 128             # K chunks = 4
MT = GH // 128            # M tiles  = 16 (4 gates x 4 cell blocks)
NCB = H // 128            # cell blocks per gate = 4
DT = 16                   # layer-pipeline delay / Zx block size
TMIN = T // 2             # smallest possible length

_BUILT = {}


def _build(t_steps=T):
    """Build the Bass module (one SPMD program for all cores)."""
    import concourse.bass as bass
    import concourse.mybir as mybir
    import concourse.tile as tile
    from concourse import bacc
    from concourse.masks import make_identity
    from contextlib import ExitStack

    f32 = mybir.dt.float32
    bf16 = mybir.dt.bfloat16
    i32 = mybir.dt.int32
    AF = mybir.ActivationFunctionType

    Tt = t_steps
    dt = min(DT, Tt)
    n_tok = Tt * BL                      # gathered rows per core
    n_tok_pad = ((n_tok + 127) // 128) * 128
    n_gblk = n_tok_pad // 128            # gather calls
    # c-history slots to emit: c at step te lives in ring slot te+1.
    # te in [Tt//2 - 1, Tt - 1] -> slots [Tt//2, Tt].
    ch0 = Tt // 2
    chn = Tt - ch0 + 1

    nc = bacc.Bacc("TRN2", target_bir_lowering=False, debug=False, num_devices=NCORES)

    toks = nc.declare_dram_parameter("tokens", [n_tok_pad], i32, isOutput=False)
    emb = nc.declare_dram_parameter("emb", [V, H], f32, isOutput=False)
    wx = nc.declare_dram_parameter("wx", [L, H, GH], f32, isOutput=False)
    wh = nc.declare_dram_parameter("wh", [L, H, GH], f32, isOutput=False)
    bia = nc.declare_dram_parameter("b", [L, GH], f32, isOutput=False)
    ys = nc.declare_dram_parameter("ys", [128, NCB, Tt, BL], f32, isOutput=True)
    chist = nc.declare_dram_parameter("chist", [128, chn, NCB * BL], f32, isOutput=True)

    with ExitStack() as ctx:
        tc = ctx.enter_context(tile.TileContext(nc))

        # ---- persistent pools -------------------------------------------
        wpool = ctx.enter_context(tc.tile_pool(name="weights", bufs=1))
        spool = ctx.enter_context(tc.tile_pool(name="state", bufs=1))
        zxpool = ctx.enter_context(tc.tile_pool(name="zx", bufs=2))
        work = ctx.enter_context(tc.tile_pool(name="work", bufs=3))

        # x^T in SBUF (bf16): [kchunk][128, n_tok_pad]
        XT = [spool.tile([128, n_tok_pad], bf16, tag=f"xt{k}", name=f"xt{k}") for k in range(KC)]

        # ---- embedding gather + transpose (prelude scope) ---------------
        with tc.tile_pool(name="gather", bufs=1) as gpool, \
             tc.tile_pool(name="tpsum", bufs=2, space="PSUM") as tpsum:
            toksb = gpool.tile([128, n_gblk], i32)
            nc.gpsimd.dma_start(out=toksb[:],
                                in_=toks.ap().rearrange("(j p) -> p j", p=128))
            xsb = [gpool.tile([128, H], f32, name=f"xsb{j}", tag=f"xsb{j}")
                   for j in range(n_gblk)]
            for j in range(n_gblk):
                nc.gpsimd.indirect_dma_start(
                    out=xsb[j][:], out_offset=None,
                    in_=emb[:, :],
                    in_offset=bass.IndirectOffsetOnAxis(ap=toksb[:, j:j + 1], axis=0),
                )
            idt = gpool.tile([128, 128], f32)
            make_identity(nc, idt[:])
            for j in range(n_gblk):
                for k in range(KC):
                    tp = tpsum.tile([128, 128], f32)
                    nc.tensor.transpose(out=tp[:], in_=xsb[j][:, 128 * k:128 * (k + 1)],
                                        identity=idt[:])
                    nc.vector.tensor_copy(out=XT[k][:, 128 * j:128 * (j + 1)], in_=tp[:])

        # Weights in SBUF, bf16, one [128, GH] tile per (layer, mat, kchunk).
        WSB = {}
        for l in range(L):
            for name, src in (("wx", wx), ("wh", wh)):
                for k in range(KC):
                    wt = wpool.tile([128, GH], bf16, tag=f"w{name}{l}{k}", name=f"w{name}{l}{k}")
                    # DMA with dtype cast (SWDGE): DRAM f32 -> SBUF bf16
                    nc.gpsimd.dma_start(out=wt[:], in_=src[l, 128 * k:128 * (k + 1), :])
                    WSB[(l, name, k)] = wt

        # bias -> [128, L, MT] cell-partition layout: bsb[p, l, m] = b[l, 128m+p]
        bsb = wpool.tile([128, L, MT], f32, tag="bias")
        nc.sync.dma_start(out=bsb[:], in_=bia.ap().rearrange("l (m p) -> p l m", p=128))

        # h history per layer: slot s holds h_{s-1}; slot 0 = zeros.
        HH = [spool.tile([128, KC, Tt + 1, BL], bf16, tag=f"h{l}", name=f"h{l}") for l in range(L)]
        for l in range(L):
            nc.vector.memset(HH[l][:, :, 0, :], 0.0)
        # gate/state tiles: [si | sf | so | tg | c], c is persistent state.
        GG = [spool.tile([128, 5 * 4 * BL], f32, tag=f"gg{l}", name=f"gg{l}")
              for l in range(L)]
        for l in range(L):
            nc.vector.memset(GG[l][:, 4 * 4 * BL:], 0.0)
        # layer-1 c history ring (slot s = c after step s-1), for chist.
        CR1 = spool.tile([128, Tt + 1, NCB * BL], f32, tag="c1", name="c1")

        # ---- hot loop ----------------------------------------------------
        zpsum = ctx.enter_context(tc.tile_pool(name="zpsum", bufs=2, space="PSUM"))
        zxpsum = ctx.enter_context(tc.tile_pool(name="zxpsum", bufs=2, space="PSUM"))

        ZXS = [[None, None] for _ in range(L)]  # double-buffered Zx blocks

        def zx_mtile(l, bi, m):
            """One M-tile of the Zx block bi for layer l (spread across ticks)."""
            t0 = bi * dt
            buf = bi % 2
            if m == 0:
                zt = zxpool.tile([128, MT, dt, BL], f32, tag=f"zx{l}",
                                 name=f"zx{l}_{bi}")
                ZXS[l][buf] = zt
            zt = ZXS[l][buf]
            corder = (0, 1, 3, 2)
            col = corder[m // NCB] * NCB + (m % NCB)
            zpx = zxpsum.tile([128, dt * BL], f32)
            for k in range(KC):
                rhs = (XT[k][:, t0 * BL:(t0 + dt) * BL] if l == 0
                       else HH[0][:, k, t0 + 1:t0 + dt + 1, :])
                nc.tensor.matmul(out=zpx[:], lhsT=WSB[(l, "wx", k)][:, 128 * m:128 * (m + 1)],
                                 rhs=rhs, start=(k == 0), stop=(k == KC - 1))
            nc.scalar.activation(zt[:, col, :, :],
                                 zpx[:].rearrange("p (t b) -> p t b", b=BL),
                                 AF.Identity, bias=bsb[:, l, m:m + 1])

        def zx_block(l, bi):
            for m in range(MT):
                zx_mtile(l, bi, m)

        def step(l, t):
            """One recurrence step of layer l (layer-1 lags by DL ticks)."""
            buf = (t // dt) % 2
            trel = t % dt
            G = 4 * BL  # columns per gate group = 16
            # psum column order [i, f, o, g] (gate 2<->3 swapped) so that
            # the three sigmoid gates are contiguous for one tanh(0.5 z) op.
            corder = (0, 1, 3, 2)
            zp = zpsum.tile([128, MT * BL], f32, tag=f"z{l}")
            # Preload Zx_t into the psum bank (off the serial chain); the
            # matmuls then accumulate onto it: the bank's has_written bits
            # were set once by the prelude dummy accumulation group and are
            # never cleared again (no start=True on this tag), so the PE
            # adds instead of overwriting.
            nc.vector.tensor_copy(zp[:].rearrange("p (m b) -> p m b", b=BL),
                                  ZXS[l][buf][:, :, trel, :])
            for m in range(MT):
                col = corder[m // NCB] * NCB + (m % NCB)
                for k in range(KC):
                    nc.tensor.matmul(out=zp[:, BL * col:BL * (col + 1)],
                                     lhsT=WSB[(l, "wh", k)][:, 128 * m:128 * (m + 1)],
                                     rhs=HH[l][:, k, t, :],
                                     start=False, stop=(k == KC - 1),
                                     skip_group_check=True)
            gt = GG[l]  # persistent [128, 5*G]: [si | sf | so | tg | c]
            # tanh for all 4 gates; i,f,o pre-scaled by 0.5 (sigmoid identity)
            nc.scalar.activation(gt[:, 0:3 * G], zp[:, 0:3 * G], AF.Tanh, scale=0.5)
            nc.scalar.activation(gt[:, 3 * G:4 * G], zp[:, 3 * G:4 * G], AF.Tanh)
            # sigmoid(x) = 0.5 tanh(x/2) + 0.5
            nc.vector.tensor_scalar(out=gt[:, 0:3 * G], in0=gt[:, 0:3 * G],
                                    scalar1=0.5, scalar2=0.5,
                                    op0=mybir.AluOpType.mult,
                                    op1=mybir.AluOpType.add)
            pp = work.tile([128, 2 * G], f32, tag=f"p{l}")
            th = work.tile([128, G], f32, tag=f"th{l}")
            # [si*tg | sf*c] in one packed mul, then fold
            nc.vector.tensor_mul(pp[:], gt[:, 0:2 * G], gt[:, 3 * G:5 * G])
            nc.vector.tensor_add(gt[:, 4 * G:5 * G], pp[:, 0:G], pp[:, G:2 * G])
            if l == 1:  # keep c history for final-state extraction
                nc.vector.tensor_copy(CR1[:, t + 1, :], gt[:, 4 * G:5 * G])
            nc.scalar.activation(th[:], gt[:, 4 * G:5 * G], AF.Tanh)
            nc.vector.tensor_mul(HH[l][:, :, t + 1, :],
                                 gt[:, 2 * G:3 * G].rearrange("p (m b) -> p m b", b=BL),
                                 th[:].rearrange("p (m b) -> p m b", b=BL))

        # one-time has_written init for the recurrent-z psum banks:
        # a start=True zero matmul per (layer, parity-buf) slot.
        zdummy = spool.tile([128, 128], bf16, tag="zdummy", name="zdummy")
        nc.vector.memset(zdummy[:], 0.0)
        for l in range(L):
            for par in range(2):
                zp0 = zpsum.tile([128, MT * BL], f32, tag=f"z{l}")
                nc.tensor.matmul(out=zp0[:], lhsT=zdummy[:],
                                 rhs=zdummy[:, 0:MT * BL], start=True, stop=True)

        zx_block(0, 0)
        n_blocks = Tt // dt
        DL = min(2 * dt, Tt)  # layer-1 delay (2 blocks, so zx1 production
        #                       finishes one block before consumption)
        for tick in range(Tt + DL):
            # spread next zx block m-tiles, one per tick, ahead of the mms
            if tick < Tt and tick // dt + 1 < n_blocks:
                zx_mtile(0, tick // dt + 1, tick % dt)
            if tick < Tt:
                step(0, tick)
            if dt <= tick < Tt + dt and (tick - dt) // dt < n_blocks:
                zx_mtile(1, (tick - dt) // dt, tick % dt)
            if tick >= DL:
                step(1, tick - DL)

        # ---- outputs -----------------------------------------------------
        # ys = h1 history slots 1..Tt, cast bf16 -> f32 in DMA (SWDGE).
        nc.gpsimd.dma_start(out=ys[:, :, :, :], in_=HH[1][:, :, 1:Tt + 1, :])
        nc.sync.dma_start(out=chist[:, :, :], in_=CR1[:, ch0:Tt + 1, :])

    nc.compile()
    return nc


def _get_nc(t_steps=T):
    if t_steps not in _BUILT:
        _BUILT[t_steps] = _build(t_steps)
    return _BUILT[t_steps]


def _prep_inputs(tokens, lengths, embedding, Wx, Wh, b, t_steps=T):
    """Build per-core in_maps."""
    tokens = np.asarray(tokens)
    n_tok = t_steps * BL
    n_tok_pad = ((n_tok + 127) // 128) * 128
    in_maps = []
    for c in range(NCORES):
        tk = np.asarray(tokens[c * BL:(c + 1) * BL, :t_steps], dtype=np.int32)
        tk_tm = np.ascontiguousarray(tk.T).reshape(-1)  # t-major flat
        tk_pad = np.zeros([n_tok_pad], np.int32)
        tk_pad[:n_tok] = tk_tm
        in_maps.append({
            "tokens": tk_pad,
            "emb": np.asarray(embedding, np.float32),
            "wx": np.asarray(Wx, np.float32),
            "wh": np.asarray(Wh, np.float32),
            "b": np.asarray(b, np.float32),
        })
    return in_maps


def _assemble(results, lengths, t_steps=T):
    """Gather per-core outputs into the full (ys, (c, h)) pytree."""
    lengths = np.asarray(lengths).astype(np.int64)
    ch0 = t_steps // 2
    ys_full = np.zeros([B, t_steps, H], np.float32)
    c_full = np.zeros([B, H], np.float32)
    h_full = np.zeros([B, H], np.float32)
    for c in range(NCORES):
        r = results[c]
        ysc = r["ys"]          # [128, NCB, Tt, BL]
        chc = r["chist"]       # [128, chn, NCB*BL]
        # ys[b, t, mc*128+p] = ysc[p, mc, t, b]
        ysc_t = np.transpose(ysc, (3, 2, 1, 0)).reshape(BL, t_steps, H)
        ys_full[c * BL:(c + 1) * BL] = ysc_t
        chc_t = chc.reshape(128, -1, NCB, BL)
        for bloc in range(BL):
            bg = c * BL + bloc
            le = int(lengths[bg])
            # tail-fill: state frozen from step le onward
            if le < t_steps:
                ys_full[bg, le:] = ys_full[bg, le - 1]
            h_full[bg] = ys_full[bg, le - 1]
            # c at step le-1 lives at ring slot le -> chist index le - ch0
            cf = chc_t[:, le - ch0, :, bloc]          # [128, NCB]
            c_full[bg] = np.transpose(cf, (1, 0)).reshape(H)
    return ys_full, (c_full, h_full)


def _install_profile_hook():
    """Provide antenv.axon_hooks in slim images so trace=True works."""
    import sys, types, ctypes, contextlib
    try:
        import antenv.axon_hooks  # noqa
        return
    except ImportError:
        pass
    so_path = "/opt/axon/libaxon_pjrt.so"
    hook = None
    try:
        lib = ctypes.CDLL(so_path)
        if hasattr(lib, "axon_start_nrt_profile"):
            lib.axon_start_nrt_profile.argtypes = [
                ctypes.POINTER(ctypes.c_int64), ctypes.c_size_t]
            lib.axon_start_nrt_profile.restype = ctypes.c_int64
            lib.axon_stop_nrt_profile.argtypes = [ctypes.c_char_p]
            lib.axon_stop_nrt_profile.restype = ctypes.c_int64

            @contextlib.contextmanager
            def _hook(output_dir, device_ids):
                import jax
                jax.devices()
                if device_ids:
                    ids = (ctypes.c_int64 * len(device_ids))(*device_ids)
                    rc = lib.axon_start_nrt_profile(ids, len(device_ids))
                else:
                    rc = lib.axon_start_nrt_profile(None, 0)
                if rc != 0:
                    raise RuntimeError(f"axon_start_nrt_profile rc={rc}")
                try:
                    yield
                finally:
                    n = lib.axon_stop_nrt_profile(str(output_dir).encode())
                    print(f"profile: {n} file(s) written to {output_dir}")
            hook = _hook
    except OSError:
        pass
    mod = types.ModuleType("antenv.axon_hooks")
    mod.get_axon_ntff_profile_hook = lambda: hook
    mod.set_axon_ntff_profile_hook = lambda h: None
    sys.modules["antenv.axon_hooks"] = mod


def run(tokens, lengths, embedding, Wx, Wh, b, t_steps=T, trace=False):
    from concourse import bass_utils
    if trace:
        _install_profile_hook()
    nc = _get_nc(t_steps)
    in_maps = _prep_inputs(tokens, lengths, embedding, Wx, Wh, b, t_steps)
    res = bass_utils.run_bass_kernel_spmd(
        nc, in_maps, core_ids=list(range(NCORES)), trace=trace)
    out = _assemble(res.results, lengths, t_steps)
    return out, res


def kernel(tokens, lengths, embedding, Wx, Wh, b):
    out, _ = run(tokens, lengths, embedding, Wx, Wh, b)
    return out


# revision 16
# speedup vs baseline: 1.2029x; 1.0000x over previous
"""Self-contained Trainium2 Bass kernel for a 2-layer length-masked LSTM encoder.

Model (matches the reference):
    x = embedding[tokens]                       # [B, T, H]
    for l in range(2): x, (c, h) = lstm_layer(x, lengths, Wx[l], Wh[l], b[l])
    return x, (c, h)

Strategy: data-parallel over batch across 8 cores (B=32 -> 4 seqs/core).
Each core runs both LSTM layers for its 4 sequences with zero cross-core
communication.  Layers are software-pipelined (layer 1 lags 32 steps) so
both layers' serial chains interleave on the engines; the per-16-step Zx
input-projection blocks are spread one M-tile per step and emitted ahead
of the recurrent matmuls so the PE has fill work while a chain waits.
Zx_t is preloaded into the PSUM bank off the critical chain (DVE copy)
and the recurrent matmuls accumulate onto it via start=False - legal
because the banks' has_written bits are set once by a prelude dummy
accumulation group and never cleared.  All four gates go through ONE
tanh table: sigmoid(x) = 0.5 tanh(x/2) + 0.5, with the 0.5 input scale
applied by the activation op and the output affine fused into a single
2x-mode tensor_scalar.

(Tensor-parallel gate splitting was evaluated: it needs a per-step
cross-core h broadcast, and neither remote_dma nor the kernel barrier is
functional under this axon PJRT runtime; ncfw collectives have a ~5us
per-call floor, so 512 calls would dominate.  Profiling: PE ~60%%
occupied, the rest is the irreducible per-step eltwise/semaphore chain
of the two layer recurrences.  Measured 1.68ms.)

Key layout trick ("cell-partition"): all per-step tensors are stored as
[128 cell-partitions, cell-block, batch] so that
  - gate M-tiles are direct [128,128] slices of the weight matrices,
  - every elementwise op pairs operands on the same partitions,
  - the h history [128, kchunk, t, b] is directly the rhs of the next
    step's matmul (no transposes in the hot loop).

Length masking is removed from the hot loop entirely: the recurrence runs
unmasked.  Frozen state at t >= len only depends on steps < len, so the
graded outputs are reconstructed afterwards (tail-fill of ys with
ys[b, len-1]; final (c, h) read from history at step len-1).
"""

import numpy as np

# Hyperparameters of the problem (hardcoded per the harness contract).
B, T, V, H, L = 32, 256, 32000, 512, 2
NCORES = 8
BL = B // NCORES          # batch per core = 4
GH = 4 * H                # gate dim = 2048
KC = H // 128             # K chunks = 4
MT = GH // 128            # M tiles  = 16 (4 gates x 4 cell blocks)
NCB = H // 128            # cell blocks per gate = 4
DT = 16                   # layer-pipeline delay / Zx block size
TMIN = T // 2             # smallest possible length

_BUILT = {}


def _build(t_steps=T):
    """Build the Bass module (one SPMD program for all cores)."""
    import concourse.bass as bass
    import concourse.mybir as mybir
    import concourse.tile as tile
    from concourse import bacc
    from concourse.masks import make_identity
    from contextlib import ExitStack

    f32 = mybir.dt.float32
    bf16 = mybir.dt.bfloat16
    i32 = mybir.dt.int32
    AF = mybir.ActivationFunctionType

    Tt = t_steps
    dt = min(DT, Tt)
    n_tok = Tt * BL                      # gathered rows per core
    n_tok_pad = ((n_tok + 127) // 128) * 128
    n_gblk = n_tok_pad // 128            # gather calls
    # c-history slots to emit: c at step te lives in ring slot te+1.
    # te in [Tt//2 - 1, Tt - 1] -> slots [Tt//2, Tt].
    ch0 = Tt // 2
    chn = Tt - ch0 + 1

    nc = bacc.Bacc("TRN2", target_bir_lowering=False, debug=False, num_devices=NCORES)

    toks = nc.declare_dram_parameter("tokens", [n_tok_pad], i32, isOutput=False)
    emb = nc.declare_dram_parameter("emb", [V, H], f32, isOutput=False)
    wx = nc.declare_dram_parameter("wx", [L, H, GH], f32, isOutput=False)
    wh = nc.declare_dram_parameter("wh", [L, H, GH], f32, isOutput=False)
    bia = nc.declare_dram_parameter("b", [L, GH], f32, isOutput=False)
    ys = nc.declare_dram_parameter("ys", [128, NCB, Tt, BL], f32, isOutput=True)
    chist = nc.declare_dram_parameter("chist", [128, chn, NCB * BL], f32, isOutput=True)

    with ExitStack() as ctx:
        tc = ctx.enter_context(tile.TileContext(nc))

        # ---- persistent pools -------------------------------------------
        wpool = ctx.enter_context(tc.tile_pool(name="weights", bufs=1))
        spool = ctx.enter_context(tc.tile_pool(name="state", bufs=1))
        zxpool = ctx.enter_context(tc.tile_pool(name="zx", bufs=2))
        work = ctx.enter_context(tc.tile_pool(name="work", bufs=3))

        # x^T in SBUF (bf16): [kchunk][128, n_tok_pad]
        XT = [spool.tile([128, n_tok_pad], bf16, tag=f"xt{k}", name=f"xt{k}") for k in range(KC)]

        # ---- embedding gather + transpose (prelude scope) ---------------
        with tc.tile_pool(name="gather", bufs=1) as gpool, \
             tc.tile_pool(name="tpsum", bufs=2, space="PSUM") as tpsum:
            toksb = gpool.tile([128, n_gblk], i32)
            nc.gpsimd.dma_start(out=toksb[:],
                                in_=toks.ap().rearrange("(j p) -> p j", p=128))
            xsb = [gpool.tile([128, H], f32, name=f"xsb{j}", tag=f"xsb{j}")
                   for j in range(n_gblk)]
            for j in range(n_gblk):
                nc.gpsimd.indirect_dma_start(
                    out=xsb[j][:], out_offset=None,
                    in_=emb[:, :],
                    in_offset=bass.IndirectOffsetOnAxis(ap=toksb[:, j:j + 1], axis=0),
                )
            idt = gpool.tile([128, 128], f32)
            make_identity(nc, idt[:])
            for j in range(n_gblk):
                for k in range(KC):
                    tp = tpsum.tile([128, 128], f32)
                    nc.tensor.transpose(out=tp[:], in_=xsb[j][:, 128 * k:128 * (k + 1)],
                                        identity=idt[:])
                    nc.vector.tensor_copy(out=XT[k][:, 128 * j:128 * (j + 1)], in_=tp[:])

        # Weights in SBUF, bf16, one [128, GH] tile per (layer, mat, kchunk).
        WSB = {}
        for l in range(L):
            for name, src in (("wx", wx), ("wh", wh)):
                for k in range(KC):
                    wt = wpool.tile([128, GH], bf16, tag=f"w{name}{l}{k}", name=f"w{name}{l}{k}")
                    # DMA with dtype cast (SWDGE): DRAM f32 -> SBUF bf16
                    nc.gpsimd.dma_start(out=wt[:], in_=src[l, 128 * k:128 * (k + 1), :])
                    WSB[(l, name, k)] = wt

        # bias -> [128, L, MT] cell-partition layout: bsb[p, l, m] = b[l, 128m+p]
        bsb = wpool.tile([128, L, MT], f32, tag="bias")
        nc.sync.dma_start(out=bsb[:], in_=bia.ap().rearrange("l (m p) -> p l m", p=128))

        # h history per layer: slot s holds h_{s-1}; slot 0 = zeros.
        HH = [spool.tile([128, KC, Tt + 1, BL], bf16, tag=f"h{l}", name=f"h{l}") for l in range(L)]
        for l in range(L):
            nc.vector.memset(HH[l][:, :, 0, :], 0.0)
        # gate/state tiles: [si | sf | so | tg | c], c is persistent state.
        GG = [spool.tile([128, 5 * 4 * BL], f32, tag=f"gg{l}", name=f"gg{l}")
              for l in range(L)]
        for l in range(L):
            nc.vector.memset(GG[l][:, 4 * 4 * BL:], 0.0)
        # layer-1 c history ring (slot s = c after step s-1), for chist.
        CR1 = spool.tile([128, Tt + 1, NCB * BL], f32, tag="c1", name="c1")

        # ---- hot loop ----------------------------------------------------
        zpsum = ctx.enter_context(tc.tile_pool(name="zpsum", bufs=2, space="PSUM"))
        zxpsum = ctx.enter_context(tc.tile_pool(name="zxpsum", bufs=2, space="PSUM"))

        ZXS = [[None, None] for _ in range(L)]  # double-buffered Zx blocks

        def zx_mtile(l, bi, m):
            """One M-tile of the Zx block bi for layer l (spread across ticks)."""
            t0 = bi * dt
            buf = bi % 2
            if m == 0:
                zt = zxpool.tile([128, MT, dt, BL], f32, tag=f"zx{l}",
                                 name=f"zx{l}_{bi}")
                ZXS[l][buf] = zt
            zt = ZXS[l][buf]
            corder = (0, 1, 3, 2)
            col = corder[m // NCB] * NCB + (m % NCB)
            zpx = zxpsum.tile([128, dt * BL], f32)
            for k in range(KC):
                rhs = (XT[k][:, t0 * BL:(t0 + dt) * BL] if l == 0
                       else HH[0][:, k, t0 + 1:t0 + dt + 1, :])
                nc.tensor.matmul(out=zpx[:], lhsT=WSB[(l, "wx", k)][:, 128 * m:128 * (m + 1)],
                                 rhs=rhs, start=(k == 0), stop=(k == KC - 1))
            nc.scalar.activation(zt[:, col, :, :],
                                 zpx[:].rearrange("p (t b) -> p t b", b=BL),
                                 AF.Identity, bias=bsb[:, l, m:m + 1])

        def zx_block(l, bi):
            for m in range(MT):
                zx_mtile(l, bi, m)

        ZPQ = [None, None]

        def zpreload(l, t):
            """Issue the Zx_t -> psum preload for step t one tick early, so
            it clears the DVE queue well before step t's matmuls need the
            bank (the matmuls accumulate onto it via start=False)."""
            if not (0 <= t < Tt):
                return
            buf = (t // dt) % 2
            trel = t % dt
            zp = zpsum.tile([128, MT * BL], f32, tag=f"z{l}", name=f"zp{l}_{t}")
            nc.vector.tensor_copy(zp[:].rearrange("p (m b) -> p m b", b=BL),
                                  ZXS[l][buf][:, :, trel, :])
            ZPQ[l] = zp

        def step(l, t):
            """One recurrence step of layer l (layer-1 lags by DL ticks)."""
            buf = (t // dt) % 2
            trel = t % dt
            G = 4 * BL  # columns per gate group = 16
            # psum column order [i, f, o, g] (gate 2<->3 swapped) so that
            # the three sigmoid gates are contiguous for one tanh(0.5 z) op.
            corder = (0, 1, 3, 2)
            zp = ZPQ[l]  # psum bank preloaded with Zx_t one tick ago
            for m in range(MT):
                col = corder[m // NCB] * NCB + (m % NCB)
                for k in range(KC):
                    nc.tensor.matmul(out=zp[:, BL * col:BL * (col + 1)],
                                     lhsT=WSB[(l, "wh", k)][:, 128 * m:128 * (m + 1)],
                                     rhs=HH[l][:, k, t, :],
                                     start=False, stop=(k == KC - 1),
                                     skip_group_check=True)
            gt = GG[l]  # persistent [128, 5*G]: [si | sf | so | tg | c]
            # tanh for all 4 gates; i,f,o pre-scaled by 0.5 (sigmoid identity)
            nc.scalar.activation(gt[:, 0:3 * G], zp[:, 0:3 * G], AF.Tanh, scale=0.5)
            nc.scalar.activation(gt[:, 3 * G:4 * G], zp[:, 3 * G:4 * G], AF.Tanh)
            # sigmoid(x) = 0.5 tanh(x/2) + 0.5
            nc.vector.tensor_scalar(out=gt[:, 0:3 * G], in0=gt[:, 0:3 * G],
                                    scalar1=0.5, scalar2=0.5,
                                    op0=mybir.AluOpType.mult,
                                    op1=mybir.AluOpType.add)
            pp = work.tile([128, 2 * G], f32, tag=f"p{l}")
            th = work.tile([128, G], f32, tag=f"th{l}")
            # [si*tg | sf*c] in one packed mul, then fold
            nc.vector.tensor_mul(pp[:], gt[:, 0:2 * G], gt[:, 3 * G:5 * G])
            nc.vector.tensor_add(gt[:, 4 * G:5 * G], pp[:, 0:G], pp[:, G:2 * G])
            if l == 1:  # keep c history for final-state extraction
                nc.vector.tensor_copy(CR1[:, t + 1, :], gt[:, 4 * G:5 * G])
            nc.scalar.activation(th[:], gt[:, 4 * G:5 * G], AF.Tanh)
            nc.vector.tensor_mul(HH[l][:, :, t + 1, :],
                                 gt[:, 2 * G:3 * G].rearrange("p (m b) -> p m b", b=BL),
                                 th[:].rearrange("p (m b) -> p m b", b=BL))

        # one-time has_written init for the recurrent-z psum banks:
        # a start=True zero matmul per (layer, parity-buf) slot.
        zdummy = spool.tile([128, 128], bf16, tag="zdummy", name="zdummy")
        nc.vector.memset(zdummy[:], 0.0)
        for l in range(L):
            for par in range(2):
                zp0 = zpsum.tile([128, MT * BL], f32, tag=f"z{l}")
                nc.tensor.matmul(out=zp0[:], lhsT=zdummy[:],
                                 rhs=zdummy[:, 0:MT * BL], start=True, stop=True)

        zx_block(0, 0)
        n_blocks = Tt // dt
        DL = min(2 * dt, Tt)  # layer-1 delay (2 blocks, so zx1 production
        #                       finishes one block before consumption)
        zpreload(0, 0)
        for tick in range(Tt + DL):
            # spread next zx block m-tiles, one per tick, ahead of the mms
            if tick < Tt and tick // dt + 1 < n_blocks:
                zx_mtile(0, tick // dt + 1, tick % dt)
            if tick < Tt:
                step(0, tick)
            zpreload(0, tick + 1)
            if dt <= tick < Tt + dt and (tick - dt) // dt < n_blocks:
                zx_mtile(1, (tick - dt) // dt, tick % dt)
            if tick >= DL:
                step(1, tick - DL)
            zpreload(1, tick - DL + 1)

        # ---- outputs -----------------------------------------------------
        # ys = h1 history slots 1..Tt, cast bf16 -> f32 in DMA (SWDGE).
        nc.gpsimd.dma_start(out=ys[:, :, :, :], in_=HH[1][:, :, 1:Tt + 1, :])
        nc.sync.dma_start(out=chist[:, :, :], in_=CR1[:, ch0:Tt + 1, :])

    nc.compile()
    return nc


def _get_nc(t_steps=T):
    if t_steps not in _BUILT:
        _BUILT[t_steps] = _build(t_steps)
    return _BUILT[t_steps]


def _prep_inputs(tokens, lengths, embedding, Wx, Wh, b, t_steps=T):
    """Build per-core in_maps."""
    tokens = np.asarray(tokens)
    n_tok = t_steps * BL
    n_tok_pad = ((n_tok + 127) // 128) * 128
    in_maps = []
    for c in range(NCORES):
        tk = np.asarray(tokens[c * BL:(c + 1) * BL, :t_steps], dtype=np.int32)
        tk_tm = np.ascontiguousarray(tk.T).reshape(-1)  # t-major flat
        tk_pad = np.zeros([n_tok_pad], np.int32)
        tk_pad[:n_tok] = tk_tm
        in_maps.append({
            "tokens": tk_pad,
            "emb": np.asarray(embedding, np.float32),
            "wx": np.asarray(Wx, np.float32),
            "wh": np.asarray(Wh, np.float32),
            "b": np.asarray(b, np.float32),
        })
    return in_maps


def _assemble(results, lengths, t_steps=T):
    """Gather per-core outputs into the full (ys, (c, h)) pytree."""
    lengths = np.asarray(lengths).astype(np.int64)
    ch0 = t_steps // 2
    ys_full = np.zeros([B, t_steps, H], np.float32)
    c_full = np.zeros([B, H], np.float32)
    h_full = np.zeros([B, H], np.float32)
    for c in range(NCORES):
        r = results[c]
        ysc = r["ys"]          # [128, NCB, Tt, BL]
        chc = r["chist"]       # [128, chn, NCB*BL]
        # ys[b, t, mc*128+p] = ysc[p, mc, t, b]
        ysc_t = np.transpose(ysc, (3, 2, 1, 0)).reshape(BL, t_steps, H)
        ys_full[c * BL:(c + 1) * BL] = ysc_t
        chc_t = chc.reshape(128, -1, NCB, BL)
        for bloc in range(BL):
            bg = c * BL + bloc
            le = int(lengths[bg])
            # tail-fill: state frozen from step le onward
            if le < t_steps:
                ys_full[bg, le:] = ys_full[bg, le - 1]
            h_full[bg] = ys_full[bg, le - 1]
            # c at step le-1 lives at ring slot le -> chist index le - ch0
            cf = chc_t[:, le - ch0, :, bloc]          # [128, NCB]
            c_full[bg] = np.transpose(cf, (1, 0)).reshape(H)
    return ys_full, (c_full, h_full)


def _install_profile_hook():
    """Provide antenv.axon_hooks in slim images so trace=True works."""
    import sys, types, ctypes, contextlib
    try:
        import antenv.axon_hooks  # noqa
        return
    except ImportError:
        pass
    so_path = "/opt/axon/libaxon_pjrt.so"
    hook = None
    try:
        lib = ctypes.CDLL(so_path)
        if hasattr(lib, "axon_start_nrt_profile"):
            lib.axon_start_nrt_profile.argtypes = [
                ctypes.POINTER(ctypes.c_int64), ctypes.c_size_t]
            lib.axon_start_nrt_profile.restype = ctypes.c_int64
            lib.axon_stop_nrt_profile.argtypes = [ctypes.c_char_p]
            lib.axon_stop_nrt_profile.restype = ctypes.c_int64

            @contextlib.contextmanager
            def _hook(output_dir, device_ids):
                import jax
                jax.devices()
                if device_ids:
                    ids = (ctypes.c_int64 * len(device_ids))(*device_ids)
                    rc = lib.axon_start_nrt_profile(ids, len(device_ids))
                else:
                    rc = lib.axon_start_nrt_profile(None, 0)
                if rc != 0:
                    raise RuntimeError(f"axon_start_nrt_profile rc={rc}")
                try:
                    yield
                finally:
                    n = lib.axon_stop_nrt_profile(str(output_dir).encode())
                    print(f"profile: {n} file(s) written to {output_dir}")
            hook = _hook
    except OSError:
        pass
    mod = types.ModuleType("antenv.axon_hooks")
    mod.get_axon_ntff_profile_hook = lambda: hook
    mod.set_axon_ntff_profile_hook = lambda h: None
    sys.modules["antenv.axon_hooks"] = mod


def run(tokens, lengths, embedding, Wx, Wh, b, t_steps=T, trace=False):
    from concourse import bass_utils
    if trace:
        _install_profile_hook()
    nc = _get_nc(t_steps)
    in_maps = _prep_inputs(tokens, lengths, embedding, Wx, Wh, b, t_steps)
    res = bass_utils.run_bass_kernel_spmd(
        nc, in_maps, core_ids=list(range(NCORES)), trace=trace)
    out = _assemble(res.results, lengths, t_steps)
    return out, res


def kernel(tokens, lengths, embedding, Wx, Wh, b):
    out, _ = run(tokens, lengths, embedding, Wx, Wh, b)
    return out


# revision 17
# speedup vs baseline: 1.2073x; 1.0036x over previous
"""Self-contained Trainium2 Bass kernel for a 2-layer length-masked LSTM encoder.

Model (matches the reference):
    x = embedding[tokens]                       # [B, T, H]
    for l in range(2): x, (c, h) = lstm_layer(x, lengths, Wx[l], Wh[l], b[l])
    return x, (c, h)

Strategy: data-parallel over batch across 8 cores (B=32 -> 4 seqs/core).
Each core runs both LSTM layers for its 4 sequences with zero cross-core
communication.  Layers are software-pipelined (layer 1 lags 32 steps) so
both layers' serial chains interleave on the engines; the per-16-step Zx
input-projection blocks are spread one M-tile per step and emitted ahead
of the recurrent matmuls so the PE has fill work while a chain waits.
Zx_t is preloaded into the PSUM bank off the critical chain (DVE copy)
and the recurrent matmuls accumulate onto it via start=False - legal
because the banks' has_written bits are set once by a prelude dummy
accumulation group and never cleared.  All four gates go through ONE
tanh table: sigmoid(x) = 0.5 tanh(x/2) + 0.5, with the 0.5 input scale
applied by the activation op and the output affine fused into a single
2x-mode tensor_scalar.

(Tensor-parallel gate splitting was evaluated: it needs a per-step
cross-core h broadcast, and neither remote_dma nor the kernel barrier is
functional under this axon PJRT runtime; ncfw collectives have a ~5us
per-call floor, so 512 calls would dominate.  Profiling: PE ~60%%
occupied, the rest is the irreducible per-step eltwise/semaphore chain
of the two layer recurrences.  Measured 1.68ms.)

Key layout trick ("cell-partition"): all per-step tensors are stored as
[128 cell-partitions, cell-block, batch] so that
  - gate M-tiles are direct [128,128] slices of the weight matrices,
  - every elementwise op pairs operands on the same partitions,
  - the h history [128, kchunk, t, b] is directly the rhs of the next
    step's matmul (no transposes in the hot loop).

Length masking is removed from the hot loop entirely: the recurrence runs
unmasked.  Frozen state at t >= len only depends on steps < len, so the
graded outputs are reconstructed afterwards (tail-fill of ys with
ys[b, len-1]; final (c, h) read from history at step len-1).
"""

import numpy as np

# Hyperparameters of the problem (hardcoded per the harness contract).
B, T, V, H, L = 32, 256, 32000, 512, 2
NCORES = 8
BL = B // NCORES          # batch per core = 4
GH = 4 * H                # gate dim = 2048
KC = H // 128             # K chunks = 4
MT = GH // 128            # M tiles  = 16 (4 gates x 4 cell blocks)
NCB = H // 128            # cell blocks per gate = 4
DT = 16                   # layer-pipeline delay / Zx block size
TMIN = T // 2             # smallest possible length

_BUILT = {}


def _build(t_steps=T):
    """Build the Bass module (one SPMD program for all cores)."""
    import concourse.bass as bass
    import concourse.mybir as mybir
    import concourse.tile as tile
    from concourse import bacc
    from concourse.masks import make_identity
    from contextlib import ExitStack

    f32 = mybir.dt.float32
    bf16 = mybir.dt.bfloat16
    i32 = mybir.dt.int32
    AF = mybir.ActivationFunctionType

    Tt = t_steps
    dt = min(DT, Tt)
    n_tok = Tt * BL                      # gathered rows per core
    n_tok_pad = ((n_tok + 127) // 128) * 128
    n_gblk = n_tok_pad // 128            # gather calls
    # c-history slots to emit: c at step te lives in ring slot te+1.
    # te in [Tt//2 - 1, Tt - 1] -> slots [Tt//2, Tt].
    ch0 = Tt // 2
    chn = Tt - ch0 + 1

    nc = bacc.Bacc("TRN2", target_bir_lowering=False, debug=False, num_devices=NCORES)

    toks = nc.declare_dram_parameter("tokens", [n_tok_pad], i32, isOutput=False)
    emb = nc.declare_dram_parameter("emb", [V, H], f32, isOutput=False)
    wx = nc.declare_dram_parameter("wx", [L, H, GH], f32, isOutput=False)
    wh = nc.declare_dram_parameter("wh", [L, H, GH], f32, isOutput=False)
    bia = nc.declare_dram_parameter("b", [L, GH], f32, isOutput=False)
    ys = nc.declare_dram_parameter("ys", [128, NCB, Tt, BL], f32, isOutput=True)
    chist = nc.declare_dram_parameter("chist", [128, chn, NCB * BL], f32, isOutput=True)

    with ExitStack() as ctx:
        tc = ctx.enter_context(tile.TileContext(nc))

        # ---- persistent pools -------------------------------------------
        wpool = ctx.enter_context(tc.tile_pool(name="weights", bufs=1))
        spool = ctx.enter_context(tc.tile_pool(name="state", bufs=1))
        zxpool = ctx.enter_context(tc.tile_pool(name="zx", bufs=2))
        work = ctx.enter_context(tc.tile_pool(name="work", bufs=3))

        # x^T in SBUF (bf16): [kchunk][128, n_tok_pad]
        XT = [spool.tile([128, n_tok_pad], bf16, tag=f"xt{k}", name=f"xt{k}") for k in range(KC)]

        # ---- embedding gather + transpose (prelude scope) ---------------
        with tc.tile_pool(name="gather", bufs=1) as gpool, \
             tc.tile_pool(name="tpsum", bufs=2, space="PSUM") as tpsum:
            toksb = gpool.tile([128, n_gblk], i32)
            nc.gpsimd.dma_start(out=toksb[:],
                                in_=toks.ap().rearrange("(j p) -> p j", p=128))
            xsb = [gpool.tile([128, H], f32, name=f"xsb{j}", tag=f"xsb{j}")
                   for j in range(n_gblk)]
            for j in range(n_gblk):
                nc.gpsimd.indirect_dma_start(
                    out=xsb[j][:], out_offset=None,
                    in_=emb[:, :],
                    in_offset=bass.IndirectOffsetOnAxis(ap=toksb[:, j:j + 1], axis=0),
                )
            idt = gpool.tile([128, 128], f32)
            make_identity(nc, idt[:])
            for j in range(n_gblk):
                for k in range(KC):
                    tp = tpsum.tile([128, 128], f32)
                    nc.tensor.transpose(out=tp[:], in_=xsb[j][:, 128 * k:128 * (k + 1)],
                                        identity=idt[:])
                    nc.vector.tensor_copy(out=XT[k][:, 128 * j:128 * (j + 1)], in_=tp[:])

        # Weights in SBUF, bf16, one [128, GH] tile per (layer, mat, kchunk).
        WSB = {}
        for l in range(L):
            for name, src in (("wx", wx), ("wh", wh)):
                for k in range(KC):
                    wt = wpool.tile([128, GH], bf16, tag=f"w{name}{l}{k}", name=f"w{name}{l}{k}")
                    # DMA with dtype cast (SWDGE): DRAM f32 -> SBUF bf16
                    nc.gpsimd.dma_start(out=wt[:], in_=src[l, 128 * k:128 * (k + 1), :])
                    WSB[(l, name, k)] = wt

        # bias -> [128, L, MT] cell-partition layout: bsb[p, l, m] = b[l, 128m+p]
        bsb = wpool.tile([128, L, MT], f32, tag="bias")
        nc.sync.dma_start(out=bsb[:], in_=bia.ap().rearrange("l (m p) -> p l m", p=128))

        # h history per layer: slot s holds h_{s-1}; slot 0 = zeros.
        HH = [spool.tile([128, KC, Tt + 1, BL], bf16, tag=f"h{l}", name=f"h{l}") for l in range(L)]
        for l in range(L):
            nc.vector.memset(HH[l][:, :, 0, :], 0.0)
        # gate/state tiles: [si | sf | so | tg | c], c is persistent state.
        GG = [spool.tile([128, 5 * 4 * BL], f32, tag=f"gg{l}", name=f"gg{l}")
              for l in range(L)]
        for l in range(L):
            nc.vector.memset(GG[l][:, 4 * 4 * BL:], 0.0)
        # layer-1 c history ring (slot s = c after step s-1), for chist.
        CR1 = spool.tile([128, Tt + 1, NCB * BL], f32, tag="c1", name="c1")

        # ---- hot loop ----------------------------------------------------
        zpsum = ctx.enter_context(tc.tile_pool(name="zpsum", bufs=3, space="PSUM"))
        zxpsum = ctx.enter_context(tc.tile_pool(name="zxpsum", bufs=2, space="PSUM"))

        ZXS = [[None, None] for _ in range(L)]  # double-buffered Zx blocks

        def zx_mtile(l, bi, m):
            """One M-tile of the Zx block bi for layer l (spread across ticks)."""
            t0 = bi * dt
            buf = bi % 2
            if m == 0:
                zt = zxpool.tile([128, MT, dt, BL], f32, tag=f"zx{l}",
                                 name=f"zx{l}_{bi}")
                ZXS[l][buf] = zt
            zt = ZXS[l][buf]
            corder = (0, 1, 3, 2)
            col = corder[m // NCB] * NCB + (m % NCB)
            zpx = zxpsum.tile([128, dt * BL], f32)
            for k in range(KC):
                rhs = (XT[k][:, t0 * BL:(t0 + dt) * BL] if l == 0
                       else HH[0][:, k, t0 + 1:t0 + dt + 1, :])
                nc.tensor.matmul(out=zpx[:], lhsT=WSB[(l, "wx", k)][:, 128 * m:128 * (m + 1)],
                                 rhs=rhs, start=(k == 0), stop=(k == KC - 1))
            nc.scalar.activation(zt[:, col, :, :],
                                 zpx[:].rearrange("p (t b) -> p t b", b=BL),
                                 AF.Identity, bias=bsb[:, l, m:m + 1])

        def zx_block(l, bi):
            for m in range(MT):
                zx_mtile(l, bi, m)

        ZPQ = [None, None]

        def zpreload(l, t):
            """Issue the Zx_t -> psum preload for step t one tick early, so
            it clears the DVE queue well before step t's matmuls need the
            bank (the matmuls accumulate onto it via start=False)."""
            if not (0 <= t < Tt):
                return
            buf = (t // dt) % 2
            trel = t % dt
            zp = zpsum.tile([128, MT * BL], f32, tag=f"z{l}", name=f"zp{l}_{t}")
            nc.vector.tensor_copy(zp[:].rearrange("p (m b) -> p m b", b=BL),
                                  ZXS[l][buf][:, :, trel, :])
            ZPQ[l] = zp

        def step(l, t):
            """One recurrence step of layer l (layer-1 lags by DL ticks)."""
            buf = (t // dt) % 2
            trel = t % dt
            G = 4 * BL  # columns per gate group = 16
            # psum column order [i, f, o, g] (gate 2<->3 swapped) so that
            # the three sigmoid gates are contiguous for one tanh(0.5 z) op.
            corder = (0, 1, 3, 2)
            zp = ZPQ[l]  # psum bank preloaded with Zx_t one tick ago
            for m in range(MT):
                col = corder[m // NCB] * NCB + (m % NCB)
                for k in range(KC):
                    nc.tensor.matmul(out=zp[:, BL * col:BL * (col + 1)],
                                     lhsT=WSB[(l, "wh", k)][:, 128 * m:128 * (m + 1)],
                                     rhs=HH[l][:, k, t, :],
                                     start=False, stop=(k == KC - 1),
                                     skip_group_check=True)
            gt = GG[l]  # persistent [128, 5*G]: [si | sf | so | tg | c]
            # tanh for all 4 gates; i,f,o pre-scaled by 0.5 (sigmoid identity)
            nc.scalar.activation(gt[:, 0:3 * G], zp[:, 0:3 * G], AF.Tanh, scale=0.5)
            nc.scalar.activation(gt[:, 3 * G:4 * G], zp[:, 3 * G:4 * G], AF.Tanh)
            # sigmoid(x) = 0.5 tanh(x/2) + 0.5
            nc.vector.tensor_scalar(out=gt[:, 0:3 * G], in0=gt[:, 0:3 * G],
                                    scalar1=0.5, scalar2=0.5,
                                    op0=mybir.AluOpType.mult,
                                    op1=mybir.AluOpType.add)
            pp = work.tile([128, 2 * G], f32, tag=f"p{l}")
            th = work.tile([128, G], f32, tag=f"th{l}")
            # [si*tg | sf*c] in one packed mul, then fold
            nc.vector.tensor_mul(pp[:], gt[:, 0:2 * G], gt[:, 3 * G:5 * G])
            nc.vector.tensor_add(gt[:, 4 * G:5 * G], pp[:, 0:G], pp[:, G:2 * G])
            if l == 1:  # keep c history for final-state extraction
                nc.vector.tensor_copy(CR1[:, t + 1, :], gt[:, 4 * G:5 * G])
            nc.scalar.activation(th[:], gt[:, 4 * G:5 * G], AF.Tanh)
            nc.vector.tensor_mul(HH[l][:, :, t + 1, :],
                                 gt[:, 2 * G:3 * G].rearrange("p (m b) -> p m b", b=BL),
                                 th[:].rearrange("p (m b) -> p m b", b=BL))

        # one-time has_written init for the recurrent-z psum banks:
        # a start=True zero matmul per (layer, parity-buf) slot.
        zdummy = spool.tile([128, 128], bf16, tag="zdummy", name="zdummy")
        nc.vector.memset(zdummy[:], 0.0)
        for l in range(L):
            for par in range(3):
                zp0 = zpsum.tile([128, MT * BL], f32, tag=f"z{l}")
                nc.tensor.matmul(out=zp0[:], lhsT=zdummy[:],
                                 rhs=zdummy[:, 0:MT * BL], start=True, stop=True)

        zx_block(0, 0)
        n_blocks = Tt // dt
        DL = min(2 * dt, Tt)  # layer-1 delay (2 blocks, so zx1 production
        #                       finishes one block before consumption)
        zpreload(0, 0)
        for tick in range(Tt + DL):
            # spread next zx block m-tiles, one per tick, ahead of the mms
            if tick < Tt and tick // dt + 1 < n_blocks:
                zx_mtile(0, tick // dt + 1, tick % dt)
            if tick < Tt:
                step(0, tick)
            zpreload(0, tick + 1)
            if dt <= tick < Tt + dt and (tick - dt) // dt < n_blocks:
                zx_mtile(1, (tick - dt) // dt, tick % dt)
            if tick >= DL:
                step(1, tick - DL)
            zpreload(1, tick - DL + 1)

        # ---- outputs -----------------------------------------------------
        # ys = h1 history slots 1..Tt, cast bf16 -> f32 in DMA (SWDGE).
        nc.gpsimd.dma_start(out=ys[:, :, :, :], in_=HH[1][:, :, 1:Tt + 1, :])
        nc.sync.dma_start(out=chist[:, :, :], in_=CR1[:, ch0:Tt + 1, :])

    nc.compile()
    return nc


def _get_nc(t_steps=T):
    if t_steps not in _BUILT:
        _BUILT[t_steps] = _build(t_steps)
    return _BUILT[t_steps]


def _prep_inputs(tokens, lengths, embedding, Wx, Wh, b, t_steps=T):
    """Build per-core in_maps."""
    tokens = np.asarray(tokens)
    n_tok = t_steps * BL
    n_tok_pad = ((n_tok + 127) // 128) * 128
    in_maps = []
    for c in range(NCORES):
        tk = np.asarray(tokens[c * BL:(c + 1) * BL, :t_steps], dtype=np.int32)
        tk_tm = np.ascontiguousarray(tk.T).reshape(-1)  # t-major flat
        tk_pad = np.zeros([n_tok_pad], np.int32)
        tk_pad[:n_tok] = tk_tm
        in_maps.append({
            "tokens": tk_pad,
            "emb": np.asarray(embedding, np.float32),
            "wx": np.asarray(Wx, np.float32),
            "wh": np.asarray(Wh, np.float32),
            "b": np.asarray(b, np.float32),
        })
    return in_maps


def _assemble(results, lengths, t_steps=T):
    """Gather per-core outputs into the full (ys, (c, h)) pytree."""
    lengths = np.asarray(lengths).astype(np.int64)
    ch0 = t_steps // 2
    ys_full = np.zeros([B, t_steps, H], np.float32)
    c_full = np.zeros([B, H], np.float32)
    h_full = np.zeros([B, H], np.float32)
    for c in range(NCORES):
        r = results[c]
        ysc = r["ys"]          # [128, NCB, Tt, BL]
        chc = r["chist"]       # [128, chn, NCB*BL]
        # ys[b, t, mc*128+p] = ysc[p, mc, t, b]
        ysc_t = np.transpose(ysc, (3, 2, 1, 0)).reshape(BL, t_steps, H)
        ys_full[c * BL:(c + 1) * BL] = ysc_t
        chc_t = chc.reshape(128, -1, NCB, BL)
        for bloc in range(BL):
            bg = c * BL + bloc
            le = int(lengths[bg])
            # tail-fill: state frozen from step le onward
            if le < t_steps:
                ys_full[bg, le:] = ys_full[bg, le - 1]
            h_full[bg] = ys_full[bg, le - 1]
            # c at step le-1 lives at ring slot le -> chist index le - ch0
            cf = chc_t[:, le - ch0, :, bloc]          # [128, NCB]
            c_full[bg] = np.transpose(cf, (1, 0)).reshape(H)
    return ys_full, (c_full, h_full)


def _install_profile_hook():
    """Provide antenv.axon_hooks in slim images so trace=True works."""
    import sys, types, ctypes, contextlib
    try:
        import antenv.axon_hooks  # noqa
        return
    except ImportError:
        pass
    so_path = "/opt/axon/libaxon_pjrt.so"
    hook = None
    try:
        lib = ctypes.CDLL(so_path)
        if hasattr(lib, "axon_start_nrt_profile"):
            lib.axon_start_nrt_profile.argtypes = [
                ctypes.POINTER(ctypes.c_int64), ctypes.c_size_t]
            lib.axon_start_nrt_profile.restype = ctypes.c_int64
            lib.axon_stop_nrt_profile.argtypes = [ctypes.c_char_p]
            lib.axon_stop_nrt_profile.restype = ctypes.c_int64

            @contextlib.contextmanager
            def _hook(output_dir, device_ids):
                import jax
                jax.devices()
                if device_ids:
                    ids = (ctypes.c_int64 * len(device_ids))(*device_ids)
                    rc = lib.axon_start_nrt_profile(ids, len(device_ids))
                else:
                    rc = lib.axon_start_nrt_profile(None, 0)
                if rc != 0:
                    raise RuntimeError(f"axon_start_nrt_profile rc={rc}")
                try:
                    yield
                finally:
                    n = lib.axon_stop_nrt_profile(str(output_dir).encode())
                    print(f"profile: {n} file(s) written to {output_dir}")
            hook = _hook
    except OSError:
        pass
    mod = types.ModuleType("antenv.axon_hooks")
    mod.get_axon_ntff_profile_hook = lambda: hook
    mod.set_axon_ntff_profile_hook = lambda h: None
    sys.modules["antenv.axon_hooks"] = mod


def run(tokens, lengths, embedding, Wx, Wh, b, t_steps=T, trace=False):
    from concourse import bass_utils
    if trace:
        _install_profile_hook()
    nc = _get_nc(t_steps)
    in_maps = _prep_inputs(tokens, lengths, embedding, Wx, Wh, b, t_steps)
    res = bass_utils.run_bass_kernel_spmd(
        nc, in_maps, core_ids=list(range(NCORES)), trace=trace)
    out = _assemble(res.results, lengths, t_steps)
    return out, res


def kernel(tokens, lengths, embedding, Wx, Wh, b):
    out, _ = run(tokens, lengths, embedding, Wx, Wh, b)
    return out


# revision 18
# speedup vs baseline: 1.2084x; 1.0009x over previous
"""Self-contained Trainium2 Bass kernel for a 2-layer length-masked LSTM encoder.

Model (matches the reference):
    x = embedding[tokens]                       # [B, T, H]
    for l in range(2): x, (c, h) = lstm_layer(x, lengths, Wx[l], Wh[l], b[l])
    return x, (c, h)

Strategy: data-parallel over batch across 8 cores (B=32 -> 4 seqs/core).
Each core runs both LSTM layers for its 4 sequences with zero cross-core
communication.  Layers are software-pipelined (layer 1 lags 32 steps) so
both layers' serial chains interleave on the engines; the per-16-step Zx
input-projection blocks are spread one M-tile per step and emitted ahead
of the recurrent matmuls so the PE has fill work while a chain waits.
Zx_t is preloaded into the PSUM bank off the critical chain (DVE copy)
and the recurrent matmuls accumulate onto it via start=False - legal
because the banks' has_written bits are set once by a prelude dummy
accumulation group and never cleared.  All four gates go through ONE
tanh table: sigmoid(x) = 0.5 tanh(x/2) + 0.5, with the 0.5 input scale
applied by the activation op and the output affine fused into a single
2x-mode tensor_scalar.

(Tensor-parallel gate splitting was evaluated: it needs a per-step
cross-core h broadcast, and neither remote_dma nor the kernel barrier is
functional under this axon PJRT runtime; ncfw collectives have a ~5us
per-call floor, so 512 calls would dominate.  Profiling: PE ~60%%
occupied, the rest is the irreducible per-step eltwise/semaphore chain
of the two layer recurrences.  Measured 1.68ms.)

Key layout trick ("cell-partition"): all per-step tensors are stored as
[128 cell-partitions, cell-block, batch] so that
  - gate M-tiles are direct [128,128] slices of the weight matrices,
  - every elementwise op pairs operands on the same partitions,
  - the h history [128, kchunk, t, b] is directly the rhs of the next
    step's matmul (no transposes in the hot loop).

Length masking is removed from the hot loop entirely: the recurrence runs
unmasked.  Frozen state at t >= len only depends on steps < len, so the
graded outputs are reconstructed afterwards (tail-fill of ys with
ys[b, len-1]; final (c, h) read from history at step len-1).
"""

import numpy as np

# Hyperparameters of the problem (hardcoded per the harness contract).
B, T, V, H, L = 32, 256, 32000, 512, 2
NCORES = 8
BL = B // NCORES          # batch per core = 4
GH = 4 * H                # gate dim = 2048
KC = H // 128             # K chunks = 4
MT = GH // 128            # M tiles  = 16 (4 gates x 4 cell blocks)
NCB = H // 128            # cell blocks per gate = 4
DT = 16                   # layer-pipeline delay / Zx block size
TMIN = T // 2             # smallest possible length

_BUILT = {}


def _build(t_steps=T):
    """Build the Bass module (one SPMD program for all cores)."""
    import concourse.bass as bass
    import concourse.mybir as mybir
    import concourse.tile as tile
    from concourse import bacc
    from concourse.masks import make_identity
    from contextlib import ExitStack

    f32 = mybir.dt.float32
    bf16 = mybir.dt.bfloat16
    i32 = mybir.dt.int32
    AF = mybir.ActivationFunctionType

    Tt = t_steps
    dt = min(DT, Tt)
    n_tok = Tt * BL                      # gathered rows per core
    n_tok_pad = ((n_tok + 127) // 128) * 128
    n_gblk = n_tok_pad // 128            # gather calls
    # c-history slots to emit: c at step te lives in ring slot te+1.
    # te in [Tt//2 - 1, Tt - 1] -> slots [Tt//2, Tt].
    ch0 = Tt // 2
    chn = Tt - ch0 + 1

    nc = bacc.Bacc("TRN2", target_bir_lowering=False, debug=False, num_devices=NCORES)

    toks = nc.declare_dram_parameter("tokens", [n_tok_pad], i32, isOutput=False)
    emb = nc.declare_dram_parameter("emb", [V, H], f32, isOutput=False)
    wx = nc.declare_dram_parameter("wx", [L, H, GH], f32, isOutput=False)
    wh = nc.declare_dram_parameter("wh", [L, H, GH], f32, isOutput=False)
    bia = nc.declare_dram_parameter("b", [L, GH], f32, isOutput=False)
    ys = nc.declare_dram_parameter("ys", [128, NCB, Tt, BL], f32, isOutput=True)
    chist = nc.declare_dram_parameter("chist", [128, chn, NCB * BL], f32, isOutput=True)

    with ExitStack() as ctx:
        tc = ctx.enter_context(tile.TileContext(nc))

        # ---- persistent pools -------------------------------------------
        wpool = ctx.enter_context(tc.tile_pool(name="weights", bufs=1))
        spool = ctx.enter_context(tc.tile_pool(name="state", bufs=1))
        zxpool = ctx.enter_context(tc.tile_pool(name="zx", bufs=2))
        work = ctx.enter_context(tc.tile_pool(name="work", bufs=4))

        # x^T in SBUF (bf16): [kchunk][128, n_tok_pad]
        XT = [spool.tile([128, n_tok_pad], bf16, tag=f"xt{k}", name=f"xt{k}") for k in range(KC)]

        # ---- embedding gather + transpose (prelude scope) ---------------
        with tc.tile_pool(name="gather", bufs=1) as gpool, \
             tc.tile_pool(name="tpsum", bufs=2, space="PSUM") as tpsum:
            toksb = gpool.tile([128, n_gblk], i32)
            nc.gpsimd.dma_start(out=toksb[:],
                                in_=toks.ap().rearrange("(j p) -> p j", p=128))
            xsb = [gpool.tile([128, H], f32, name=f"xsb{j}", tag=f"xsb{j}")
                   for j in range(n_gblk)]
            for j in range(n_gblk):
                nc.gpsimd.indirect_dma_start(
                    out=xsb[j][:], out_offset=None,
                    in_=emb[:, :],
                    in_offset=bass.IndirectOffsetOnAxis(ap=toksb[:, j:j + 1], axis=0),
                )
            idt = gpool.tile([128, 128], f32)
            make_identity(nc, idt[:])
            for j in range(n_gblk):
                for k in range(KC):
                    tp = tpsum.tile([128, 128], f32)
                    nc.tensor.transpose(out=tp[:], in_=xsb[j][:, 128 * k:128 * (k + 1)],
                                        identity=idt[:])
                    nc.vector.tensor_copy(out=XT[k][:, 128 * j:128 * (j + 1)], in_=tp[:])

        # Weights in SBUF, bf16, one [128, GH] tile per (layer, mat, kchunk).
        WSB = {}
        for l in range(L):
            for name, src in (("wx", wx), ("wh", wh)):
                for k in range(KC):
                    wt = wpool.tile([128, GH], bf16, tag=f"w{name}{l}{k}", name=f"w{name}{l}{k}")
                    # DMA with dtype cast (SWDGE): DRAM f32 -> SBUF bf16
                    nc.gpsimd.dma_start(out=wt[:], in_=src[l, 128 * k:128 * (k + 1), :])
                    WSB[(l, name, k)] = wt

        # bias -> [128, L, MT] cell-partition layout: bsb[p, l, m] = b[l, 128m+p]
        bsb = wpool.tile([128, L, MT], f32, tag="bias")
        nc.sync.dma_start(out=bsb[:], in_=bia.ap().rearrange("l (m p) -> p l m", p=128))

        # h history per layer: slot s holds h_{s-1}; slot 0 = zeros.
        HH = [spool.tile([128, KC, Tt + 1, BL], bf16, tag=f"h{l}", name=f"h{l}") for l in range(L)]
        for l in range(L):
            nc.vector.memset(HH[l][:, :, 0, :], 0.0)
        # gate/state tiles: [si | sf | so | tg | c], c is persistent state.
        GG = [spool.tile([128, 5 * 4 * BL], f32, tag=f"gg{l}", name=f"gg{l}")
              for l in range(L)]
        for l in range(L):
            nc.vector.memset(GG[l][:, 4 * 4 * BL:], 0.0)
        # layer-1 c history ring (slot s = c after step s-1), for chist.
        CR1 = spool.tile([128, Tt + 1, NCB * BL], f32, tag="c1", name="c1")

        # ---- hot loop ----------------------------------------------------
        zpsum = ctx.enter_context(tc.tile_pool(name="zpsum", bufs=3, space="PSUM"))
        zxpsum = ctx.enter_context(tc.tile_pool(name="zxpsum", bufs=2, space="PSUM"))

        ZXS = [[None, None] for _ in range(L)]  # double-buffered Zx blocks

        def zx_mtile(l, bi, m):
            """One M-tile of the Zx block bi for layer l (spread across ticks)."""
            t0 = bi * dt
            buf = bi % 2
            if m == 0:
                zt = zxpool.tile([128, MT, dt, BL], f32, tag=f"zx{l}",
                                 name=f"zx{l}_{bi}")
                ZXS[l][buf] = zt
            zt = ZXS[l][buf]
            corder = (0, 1, 3, 2)
            col = corder[m // NCB] * NCB + (m % NCB)
            zpx = zxpsum.tile([128, dt * BL], f32)
            for k in range(KC):
                rhs = (XT[k][:, t0 * BL:(t0 + dt) * BL] if l == 0
                       else HH[0][:, k, t0 + 1:t0 + dt + 1, :])
                nc.tensor.matmul(out=zpx[:], lhsT=WSB[(l, "wx", k)][:, 128 * m:128 * (m + 1)],
                                 rhs=rhs, start=(k == 0), stop=(k == KC - 1))
            nc.scalar.activation(zt[:, col, :, :],
                                 zpx[:].rearrange("p (t b) -> p t b", b=BL),
                                 AF.Identity, bias=bsb[:, l, m:m + 1])

        def zx_block(l, bi):
            for m in range(MT):
                zx_mtile(l, bi, m)

        ZPQ = [None, None]

        def zpreload(l, t):
            """Issue the Zx_t -> psum preload for step t one tick early, so
            it clears the DVE queue well before step t's matmuls need the
            bank (the matmuls accumulate onto it via start=False)."""
            if not (0 <= t < Tt):
                return
            buf = (t // dt) % 2
            trel = t % dt
            zp = zpsum.tile([128, MT * BL], f32, tag=f"z{l}", name=f"zp{l}_{t}")
            nc.vector.tensor_copy(zp[:].rearrange("p (m b) -> p m b", b=BL),
                                  ZXS[l][buf][:, :, trel, :])
            ZPQ[l] = zp

        def step(l, t):
            """One recurrence step of layer l (layer-1 lags by DL ticks)."""
            buf = (t // dt) % 2
            trel = t % dt
            G = 4 * BL  # columns per gate group = 16
            # psum column order [i, f, o, g] (gate 2<->3 swapped) so that
            # the three sigmoid gates are contiguous for one tanh(0.5 z) op.
            corder = (0, 1, 3, 2)
            zp = ZPQ[l]  # psum bank preloaded with Zx_t one tick ago
            for m in range(MT):
                col = corder[m // NCB] * NCB + (m % NCB)
                for k in range(KC):
                    nc.tensor.matmul(out=zp[:, BL * col:BL * (col + 1)],
                                     lhsT=WSB[(l, "wh", k)][:, 128 * m:128 * (m + 1)],
                                     rhs=HH[l][:, k, t, :],
                                     start=False, stop=(k == KC - 1),
                                     skip_group_check=True)
            gt = GG[l]  # persistent [128, 5*G]: [si | sf | so | tg | c]
            # tanh for all 4 gates; i,f,o pre-scaled by 0.5 (sigmoid identity)
            nc.scalar.activation(gt[:, 0:3 * G], zp[:, 0:3 * G], AF.Tanh, scale=0.5)
            nc.scalar.activation(gt[:, 3 * G:4 * G], zp[:, 3 * G:4 * G], AF.Tanh)
            # sigmoid(x) = 0.5 tanh(x/2) + 0.5
            nc.vector.tensor_scalar(out=gt[:, 0:3 * G], in0=gt[:, 0:3 * G],
                                    scalar1=0.5, scalar2=0.5,
                                    op0=mybir.AluOpType.mult,
                                    op1=mybir.AluOpType.add)
            pp = work.tile([128, 2 * G], f32, tag=f"p{l}")
            th = work.tile([128, G], f32, tag=f"th{l}")
            # [si*tg | sf*c] in one packed mul, then fold
            nc.vector.tensor_mul(pp[:], gt[:, 0:2 * G], gt[:, 3 * G:5 * G])
            nc.vector.tensor_add(gt[:, 4 * G:5 * G], pp[:, 0:G], pp[:, G:2 * G])
            if l == 1:  # keep c history for final-state extraction
                nc.vector.tensor_copy(CR1[:, t + 1, :], gt[:, 4 * G:5 * G])
            nc.scalar.activation(th[:], gt[:, 4 * G:5 * G], AF.Tanh)
            nc.vector.tensor_mul(HH[l][:, :, t + 1, :],
                                 gt[:, 2 * G:3 * G].rearrange("p (m b) -> p m b", b=BL),
                                 th[:].rearrange("p (m b) -> p m b", b=BL))

        # one-time has_written init for the recurrent-z psum banks:
        # a start=True zero matmul per (layer, parity-buf) slot.
        zdummy = spool.tile([128, 128], bf16, tag="zdummy", name="zdummy")
        nc.vector.memset(zdummy[:], 0.0)
        for l in range(L):
            for par in range(3):
                zp0 = zpsum.tile([128, MT * BL], f32, tag=f"z{l}")
                nc.tensor.matmul(out=zp0[:], lhsT=zdummy[:],
                                 rhs=zdummy[:, 0:MT * BL], start=True, stop=True)

        zx_block(0, 0)
        n_blocks = Tt // dt
        DL = min(2 * dt, Tt)  # layer-1 delay (2 blocks, so zx1 production
        #                       finishes one block before consumption)
        zpreload(0, 0)
        for tick in range(Tt + DL):
            # spread next zx block m-tiles, one per tick, ahead of the mms
            if tick < Tt and tick // dt + 1 < n_blocks:
                zx_mtile(0, tick // dt + 1, tick % dt)
            if tick < Tt:
                step(0, tick)
            zpreload(0, tick + 1)
            if dt <= tick < Tt + dt and (tick - dt) // dt < n_blocks:
                zx_mtile(1, (tick - dt) // dt, tick % dt)
            if tick >= DL:
                step(1, tick - DL)
            zpreload(1, tick - DL + 1)

        # ---- outputs -----------------------------------------------------
        # ys = h1 history slots 1..Tt, cast bf16 -> f32 in DMA (SWDGE).
        nc.gpsimd.dma_start(out=ys[:, :, :, :], in_=HH[1][:, :, 1:Tt + 1, :])
        nc.sync.dma_start(out=chist[:, :, :], in_=CR1[:, ch0:Tt + 1, :])

    nc.compile()
    return nc


def _get_nc(t_steps=T):
    if t_steps not in _BUILT:
        _BUILT[t_steps] = _build(t_steps)
    return _BUILT[t_steps]


def _prep_inputs(tokens, lengths, embedding, Wx, Wh, b, t_steps=T):
    """Build per-core in_maps."""
    tokens = np.asarray(tokens)
    n_tok = t_steps * BL
    n_tok_pad = ((n_tok + 127) // 128) * 128
    in_maps = []
    for c in range(NCORES):
        tk = np.asarray(tokens[c * BL:(c + 1) * BL, :t_steps], dtype=np.int32)
        tk_tm = np.ascontiguousarray(tk.T).reshape(-1)  # t-major flat
        tk_pad = np.zeros([n_tok_pad], np.int32)
        tk_pad[:n_tok] = tk_tm
        in_maps.append({
            "tokens": tk_pad,
            "emb": np.asarray(embedding, np.float32),
            "wx": np.asarray(Wx, np.float32),
            "wh": np.asarray(Wh, np.float32),
            "b": np.asarray(b, np.float32),
        })
    return in_maps


def _assemble(results, lengths, t_steps=T):
    """Gather per-core outputs into the full (ys, (c, h)) pytree."""
    lengths = np.asarray(lengths).astype(np.int64)
    ch0 = t_steps // 2
    ys_full = np.zeros([B, t_steps, H], np.float32)
    c_full = np.zeros([B, H], np.float32)
    h_full = np.zeros([B, H], np.float32)
    for c in range(NCORES):
        r = results[c]
        ysc = r["ys"]          # [128, NCB, Tt, BL]
        chc = r["chist"]       # [128, chn, NCB*BL]
        # ys[b, t, mc*128+p] = ysc[p, mc, t, b]
        ysc_t = np.transpose(ysc, (3, 2, 1, 0)).reshape(BL, t_steps, H)
        ys_full[c * BL:(c + 1) * BL] = ysc_t
        chc_t = chc.reshape(128, -1, NCB, BL)
        for bloc in range(BL):
            bg = c * BL + bloc
            le = int(lengths[bg])
            # tail-fill: state frozen from step le onward
            if le < t_steps:
                ys_full[bg, le:] = ys_full[bg, le - 1]
            h_full[bg] = ys_full[bg, le - 1]
            # c at step le-1 lives at ring slot le -> chist index le - ch0
            cf = chc_t[:, le - ch0, :, bloc]          # [128, NCB]
            c_full[bg] = np.transpose(cf, (1, 0)).reshape(H)
    return ys_full, (c_full, h_full)


def _install_profile_hook():
    """Provide antenv.axon_hooks in slim images so trace=True works."""
    import sys, types, ctypes, contextlib
    try:
        import antenv.axon_hooks  # noqa
        return
    except ImportError:
        pass
    so_path = "/opt/axon/libaxon_pjrt.so"
    hook = None
    try:
        lib = ctypes.CDLL(so_path)
        if hasattr(lib, "axon_start_nrt_profile"):
            lib.axon_start_nrt_profile.argtypes = [
                ctypes.POINTER(ctypes.c_int64), ctypes.c_size_t]
            lib.axon_start_nrt_profile.restype = ctypes.c_int64
            lib.axon_stop_nrt_profile.argtypes = [ctypes.c_char_p]
            lib.axon_stop_nrt_profile.restype = ctypes.c_int64

            @contextlib.contextmanager
            def _hook(output_dir, device_ids):
                import jax
                jax.devices()
                if device_ids:
                    ids = (ctypes.c_int64 * len(device_ids))(*device_ids)
                    rc = lib.axon_start_nrt_profile(ids, len(device_ids))
                else:
                    rc = lib.axon_start_nrt_profile(None, 0)
                if rc != 0:
                    raise RuntimeError(f"axon_start_nrt_profile rc={rc}")
                try:
                    yield
                finally:
                    n = lib.axon_stop_nrt_profile(str(output_dir).encode())
                    print(f"profile: {n} file(s) written to {output_dir}")
            hook = _hook
    except OSError:
        pass
    mod = types.ModuleType("antenv.axon_hooks")
    mod.get_axon_ntff_profile_hook = lambda: hook
    mod.set_axon_ntff_profile_hook = lambda h: None
    sys.modules["antenv.axon_hooks"] = mod


def run(tokens, lengths, embedding, Wx, Wh, b, t_steps=T, trace=False):
    from concourse import bass_utils
    if trace:
        _install_profile_hook()
    nc = _get_nc(t_steps)
    in_maps = _prep_inputs(tokens, lengths, embedding, Wx, Wh, b, t_steps)
    res = bass_utils.run_bass_kernel_spmd(
        nc, in_maps, core_ids=list(range(NCORES)), trace=trace)
    out = _assemble(res.results, lengths, t_steps)
    return out, res


def kernel(tokens, lengths, embedding, Wx, Wh, b):
    out, _ = run(tokens, lengths, embedding, Wx, Wh, b)
    return out
